# revision 1
# baseline (speedup 1.0000x reference)
"""Trainium2 Bass kernel for nn_MultiHeadAttn (dense transformer block).

Contract: kernel(**inputs) takes the FULL unsharded inputs from
reference.setup_inputs() and returns the FULL output [8, 1024, 768] f32.

Sharding: data-parallel over batch N=8 -> one batch item per NeuronCore
(8 cores), no collectives.

Per-core design (all activations kept transposed: channels on partitions,
sequence on the free dim; host does the boundary transposes for free):
  xT [768,1024] -> qT/kT [hid,seq] via blockdiag weights (2 heads/tile),
  v natural [seq,hid] via blockdiag, scoresT = kT_slice.T @ qT (float32r),
  exp on ScalarE (PSUM drain) -> f32r, AV with ones-column in lhsT (M=65)
  giving per-q rowsums for free, per-head proj with col-group packing,
  softmax normalization after proj (linearity), LN stats via ones-matmul
  over partitions, FF matmul, residual add.

Channel permutation: attention output channels are produced head-major
(c' = h*64+dh) while the module interleaves (c = dh*12+h). LN is
permutation-invariant; wff rows/cols, bff, gamma, beta are permuted on
the host and the final output is unpermuted on the host.
"""

import numpy as np

import concourse.bacc as bacc
import concourse.mybir as mybir
import concourse.tile as tile
from concourse.bass_utils import run_bass_kernel_spmd

F32 = mybir.dt.float32
F32R = mybir.dt.float32r
AF = mybir.ActivationFunctionType
OP = mybir.AluOpType

S = 1024  # sequence length
D = 768  # model dim
H = 12  # heads
DH = 64  # head dim
HID = 64  # per-head hidden
NT = 6  # channel tiles of 128 (2 heads each)
KC = 8  # key chunks of 128
LN_EPS = 1e-5

_CACHE = {}


def build_nc(loop_n=None, debug=False, phases='all'):
    """Build the single-core bass program (SPMD across 8 cores)."""
    nc = bacc.Bacc("TRN2", target_bir_lowering=False, debug=False)

    xT_d = nc.dram_tensor("xT", [D, S], F32R, kind="ExternalInput")
    wq2_d = nc.dram_tensor("wq2", [128, 128], F32R, kind="ExternalInput")
    wkA2_d = nc.dram_tensor("wkA2", [128, 128], F32R, kind="ExternalInput")
    wkB2_d = nc.dram_tensor("wkB2", [128, 128], F32R, kind="ExternalInput")
    wv2b_d = nc.dram_tensor("wv2b", [128, 130], F32R, kind="ExternalInput")
    wp_d = nc.dram_tensor("wp2", [128, 128], F32R, kind="ExternalInput")
    bq2_d = nc.dram_tensor("bq2", [128, 1], F32, kind="ExternalInput")
    bp2_d = nc.dram_tensor("bp2", [128, 1], F32, kind="ExternalInput")
    wffp_d = nc.dram_tensor("wffp", [D, D], F32R, kind="ExternalInput")
    bffp_d = nc.dram_tensor("bffp", [128, NT], F32, kind="ExternalInput")
    ones_d = nc.dram_tensor("ones", [128, 1], F32R, kind="ExternalInput")
    out_d = nc.dram_tensor("out", [D, S], F32, kind="ExternalOutput")
    dbg = {}
    if debug:
        for nm, shp in [
            ("dbg_qT", [128, S]), ("dbg_kT", [128, S]), ("dbg_v", [128, 130]),
            ("dbg_eA", [128, S]), ("dbg_uAs", [65, S]), ("dbg_u2", [128, S]),
            ("dbg_rb2", [128, S]), ("dbg_a", [128, S]), ("dbg_mean", [1, S]),
            ("dbg_rstd", [1, S]), ("dbg_nr", [128, S]),
        ]:
            dbg[nm] = nc.dram_tensor(nm, shp, F32, kind="ExternalOutput")

    with tile.TileContext(nc) as tc:

        def body(_i=None):
            with (
                tc.tile_pool(name="const", bufs=1) as cpool,
                tc.tile_pool(name="atile", bufs=1) as apool,
                tc.tile_pool(name="psS", bufs=2, space="PSUM") as psS,
                tc.tile_pool(name="psU", bufs=2, space="PSUM") as psU,
            ):
                # ---- constants / weights (f32r DMAed directly) ----
                def load(dram, shape, dt=F32R):
                    r = cpool.tile(shape, dt, name=f"r_{dram.name}")
                    nc.sync.dma_start(r[:], dram[:])
                    return r

                wq2r = load(wq2_d, [128, 128])
                wkA2r = load(wkA2_d, [128, 128])
                wkB2r = load(wkB2_d, [128, 128])
                wv2br = load(wv2b_d, [128, 130])
                wpr = load(wp_d, [128, 128])
                onesr = load(ones_d, [128, 1])
                bq2 = load(bq2_d, [128, 1], F32)
                bp2 = load(bp2_d, [128, 1], F32)
                wffr = []
                for t in range(NT):
                    r = cpool.tile([128, D], F32R, name=f"wffr{t}")
                    nc.sync.dma_start(r[:], wffp_d[128 * t : 128 * (t + 1), :])
                    wffr.append(r)
                bff_all = cpool.tile([128, NT], F32, name="bff_all")
                nc.sync.dma_start(bff_all[:], bffp_d[:])
                gb = [bff_all[:, t : t + 1] for t in range(NT)]

                aT = []
                with (
                    tc.tile_pool(name="qkv", bufs=1) as qkvpool,
                    tc.tile_pool(name="xr", bufs=1) as xrpool,
                    tc.tile_pool(name="p2w", bufs=2) as w2,
                ):
                    qTr = [None] * NT
                    kTr = [None] * NT
                    v2r = [None] * NT

                    def proj_pair(t):
                        xr = xrpool.tile(
                            [128, S], F32R, name="xr", tag="xr", bufs=2
                        )
                        nc.sync.dma_start(xr[:], xT_d[128 * t : 128 * (t + 1), :])
                        ps = psS.tile([128, S], F32, name="qkps", tag="s")
                        for qh in range(2):
                            nc.tensor.matmul(
                                ps[:, 512 * qh : 512 * (qh + 1)],
                                wq2r[:],
                                xr[:, 512 * qh : 512 * (qh + 1)],
                                start=True,
                                stop=True,
                            )
                        q = qkvpool.tile([128, S], F32R, name=f"qTr{t}")
                        nc.vector.tensor_scalar_add(q[:], ps[:], bq2[:])
                        qTr[t] = q
                        _kk = []
                        for wkr in (wkA2r, wkB2r):
                            ps2 = psS.tile([128, S], F32, name="qkps", tag="s")
                            for qh in range(2):
                                nc.tensor.matmul(
                                    ps2[:, 512 * qh : 512 * (qh + 1)],
                                    wkr[:],
                                    xr[:, 512 * qh : 512 * (qh + 1)],
                                    start=True,
                                    stop=True,
                                )
                            kk = qkvpool.tile(
                                [128, S], F32R, name=f"kTr{t}_{len(_kk)}"
                            )
                            nc.vector.tensor_copy(kk[:], ps2[:])
                            _kk.append(kk)
                        kTr[t] = _kk
                        vs = qkvpool.tile([128, 130 * KC], F32R, name=f"v2r{t}")
                        for scp in range(KC // 2):
                            vps = psS.tile([128, 260], F32, name="vps", tag="s")
                            for j in range(2):
                                nc.tensor.matmul(
                                    vps[:, 130 * j : 130 * (j + 1)],
                                    xr[:, 128 * (2 * scp + j) : 128 * (2 * scp + j + 1)],
                                    wv2br[:],
                                    start=True,
                                    stop=True,
                                )
                            nc.vector.tensor_copy(
                                vs[:, 260 * scp : 260 * (scp + 1)], vps[:]
                            )
                        for sc in range(KC):
                            nc.vector.tensor_copy(vs[:, 130 * sc : 130 * sc + 1], onesr[:])
                            nc.vector.tensor_copy(
                                vs[:, 130 * sc + 65 : 130 * sc + 66], onesr[:]
                            )
                        v2r[t] = vs
                        if debug and t == 0:
                            nc.sync.dma_start(
                                dbg["dbg_qT"][:], qTr[0][:].bitcast(F32)
                            )
                            nc.sync.dma_start(
                                dbg["dbg_kT"][0:64, :], kTr[0][0][0:64, :].bitcast(F32)
                            )
                            nc.sync.dma_start(
                                dbg["dbg_kT"][64:128, :], kTr[0][1][64:128, :].bitcast(F32)
                            )
                            nc.sync.dma_start(
                                dbg["dbg_v"][:], v2r[0][:, 0:130].bitcast(F32)
                            )

                    proj_pair(0)
                    for t in range(NT):
                        uA = psU.tile([65, S], F32, name="uA", tag="u")
                        uB = psU.tile([65, S], F32, name="uB", tag="u")
                        for kc in range(KC):
                            sA = psS.tile([128, S], F32, name="sA", tag="s")
                            sB = psS.tile([128, S], F32, name="sB", tag="s")
                            for qh in range(2):
                                nc.tensor.matmul(
                                    sA[:, 512 * qh : 512 * (qh + 1)],
                                    kTr[t][0][:, 128 * kc : 128 * (kc + 1)],
                                    qTr[t][:, 512 * qh : 512 * (qh + 1)],
                                    start=True,
                                    stop=True,
                                )
                            for qh in range(2):
                                nc.tensor.matmul(
                                    sB[:, 512 * qh : 512 * (qh + 1)],
                                    kTr[t][1][:, 128 * kc : 128 * (kc + 1)],
                                    qTr[t][:, 512 * qh : 512 * (qh + 1)],
                                    start=True,
                                    stop=True,
                                )
                            eA = w2.tile([128, S], F32R, name="eA", tag="eA", bufs=3)
                            nc.scalar.activation(eA[:], sA[:], AF.Exp)
                            if debug and t == 0 and kc == 0:
                                nc.sync.dma_start(
                                    dbg["dbg_eA"][:], eA[:].bitcast(F32)
                                )
                            eB = w2.tile([128, S], F32R, name="eB", tag="eB", bufs=2)
                            nc.scalar.activation(eB[:], sB[:], AF.Exp)
                            st = kc == 0
                            fin = kc == KC - 1
                            for qh in range(2):
                                nc.tensor.matmul(
                                    uA[:, 512 * qh : 512 * (qh + 1)],
                                    v2r[t][:, 130 * kc : 130 * kc + 65],
                                    eA[:, 512 * qh : 512 * (qh + 1)],
                                    start=st,
                                    stop=fin,
                                )
                            for qh in range(2):
                                nc.tensor.matmul(
                                    uB[:, 512 * qh : 512 * (qh + 1)],
                                    v2r[t][:, 130 * kc + 65 : 130 * (kc + 1)],
                                    eB[:, 512 * qh : 512 * (qh + 1)],
                                    start=st,
                                    stop=fin,
                                )
                            # prefetch next pair's projections mid-loop so
                            # kT/qT/v are ready before this pair's tail
                            if kc == 3 and t + 1 < NT:
                                proj_pair(t + 1)
                        # allocate p2's psum slot now so the next pair's
                        # scores aren't blocked behind it in slot rotation
                        p2 = psS.tile([128, S], F32, name="p2", tag="s")
                        # drain U (row 64 holds exp-rowsums)
                        uAs = w2.tile([65, S], F32R, name="uAs", tag="uAs")
                        nc.vector.tensor_copy(uAs[:], uA[:])
                        uBs = w2.tile([65, S], F32R, name="uBs", tag="uBs")
                        nc.vector.tensor_copy(uBs[:], uB[:])
                        if debug and t == 0:
                            nc.sync.dma_start(
                                dbg["dbg_uAs"][:], uAs[:].bitcast(F32)
                            )
                        nc.vector.reciprocal(
                            uAs[0:1, :].bitcast(F32), uAs[0:1, :].bitcast(F32)
                        )
                        nc.vector.reciprocal(
                            uBs[0:1, :].bitcast(F32), uBs[0:1, :].bitcast(F32)
                        )
                        rb2a = w2.tile([128, S], F32, name="rb2a", tag="rb2a", bufs=1)
                        nc.gpsimd.partition_broadcast(
                            rb2a[:], uAs[0:1, :].bitcast(F32)
                        )
                        rb2b = w2.tile([128, S], F32, name="rb2b", tag="rb2b", bufs=1)
                        nc.gpsimd.partition_broadcast(
                            rb2b[:], uBs[0:1, :].bitcast(F32)
                        )
                        # stack U_A / U_B (DMA shifts partitions)
                        u2 = w2.tile([128, S], F32R, name="u2", tag="u2", bufs=1)
                        nc.gpsimd.dma_start(u2[0:64, :], uAs[1:65, :])
                        nc.gpsimd.dma_start(u2[64:128, :], uBs[1:65, :])
                        if debug and t == 0:
                            nc.sync.dma_start(
                                dbg["dbg_u2"][:], u2[:].bitcast(F32)
                            )
                            nc.sync.dma_start(
                                dbg["dbg_rb2"][0:64, :], rb2a[0:64, :]
                            )
                            nc.sync.dma_start(
                                dbg["dbg_rb2"][64:128, :], rb2b[64:128, :]
                            )
                        # per-head output proj via blockdiag wp2
                        for qh in range(2):
                            nc.tensor.matmul(
                                p2[:, 512 * qh : 512 * (qh + 1)],
                                wpr[:],
                                u2[:, 512 * qh : 512 * (qh + 1)],
                                start=True,
                                stop=True,
                            )
                        # A'_t = p2 * (1/rowsum) + bp2'
                        a1 = w2.tile([128, S], F32, name="a1", tag="a1")
                        nc.vector.tensor_mul(a1[0:64, :], p2[0:64, :], rb2a[0:64, :])
                        nc.vector.tensor_mul(
                            a1[64:128, :], p2[64:128, :], rb2b[64:128, :]
                        )
                        at = apool.tile([128, S], F32R, name=f"aT{t}")
                        nc.vector.tensor_scalar_add(at[:], a1[:], bp2[:])
                        if debug and t == 0:
                            nc.sync.dma_start(dbg["dbg_a"][:], at[:].bitcast(F32))
                        aT.append(at)

                if phases == 'attn':
                    for t in range(NT):
                        nc.sync.dma_start(
                            out_d[128 * t : 128 * (t + 1), :],
                            aT[t][:].bitcast(F32),
                        )
                    return
                # ---- phase 3: LN, FF, residual ----
                with tc.tile_pool(name="p3w", bufs=2) as w3, \
                     tc.tile_pool(name="p3s", bufs=1) as s3:
                    sums = psU.tile([1, S], F32, name="sums", tag="u")
                    sumsq = psU.tile([1, S], F32, name="sumsq", tag="u")
                    for t in range(NT):
                        sq = w3.tile([128, S], F32R, name="sq", tag="sq")
                        nc.scalar.square(sq[:], aT[t][:])
                        st = t == 0
                        fin = t == NT - 1
                        for qh in range(2):
                            nc.tensor.matmul(
                                sums[:, 512 * qh : 512 * (qh + 1)],
                                onesr[:],
                                aT[t][:, 512 * qh : 512 * (qh + 1)],
                                start=st,
                                stop=fin,
                            )
                            nc.tensor.matmul(
                                sumsq[:, 512 * qh : 512 * (qh + 1)],
                                onesr[:],
                                sq[:, 512 * qh : 512 * (qh + 1)],
                                start=st,
                                stop=fin,
                            )
                    mean = s3.tile([1, S], F32, name="mean")
                    nc.vector.tensor_scalar_mul(mean[:], sums[:], 1.0 / D)
                    msq = s3.tile([1, S], F32, name="msq")
                    nc.vector.tensor_scalar_mul(msq[:], sumsq[:], 1.0 / D)
                    m2 = s3.tile([1, S], F32, name="m2")
                    nc.vector.tensor_mul(m2[:], mean[:], mean[:])
                    vpe = s3.tile([1, S], F32, name="vpe")
                    nc.vector.scalar_tensor_tensor(
                        vpe[:], msq[:], LN_EPS, m2[:], op0=OP.add, op1=OP.subtract
                    )
                    std = s3.tile([1, S], F32, name="std")
                    nc.scalar.sqrt(std[:], vpe[:])
                    rstd = s3.tile([1, S], F32, name="rstd")
                    nc.vector.reciprocal(rstd[:], std[:])
                    if debug:
                        nc.sync.dma_start(dbg["dbg_mean"][:], mean[:])
                        nc.sync.dma_start(dbg["dbg_rstd"][:], rstd[:])
                    meanB = s3.tile([128, S], F32, name="meanB")
                    nc.gpsimd.partition_broadcast(meanB[:], mean[:])
                    rstdB = s3.tile([128, S], F32, name="rstdB")
                    nc.gpsimd.partition_broadcast(rstdB[:], rstd[:])

                    normedr = []
                    for t in range(NT):
                        d0 = w3.tile([128, S], F32, name="d0", tag="d0")
                        nc.vector.tensor_sub(d0[:], aT[t][:], meanB[:])
                        nr = s3.tile([128, S], F32R, name=f"nr{t}")
                        nc.vector.tensor_mul(nr[:], d0[:], rstdB[:])
                        if debug and t == 0:
                            nc.sync.dma_start(
                                dbg["dbg_nr"][:], nr[:].bitcast(F32)
                            )
                        normedr.append(nr)

                    for m in range(NT):
                        ff = psS.tile([128, S], F32, name="ff", tag="s")
                        for kc in range(NT):
                            st = kc == 0
                            fin = kc == NT - 1
                            for qh in range(2):
                                nc.tensor.matmul(
                                    ff[:, 512 * qh : 512 * (qh + 1)],
                                    wffr[kc][:, 128 * m : 128 * (m + 1)],
                                    normedr[kc][:, 512 * qh : 512 * (qh + 1)],
                                    start=st,
                                    stop=fin,
                                )
                        y = w3.tile([128, S], F32, name="y", tag="y")
                        nc.vector.scalar_tensor_tensor(
                            y[:], ff[:], gb[m], aT[m][:],
                            op0=OP.add, op1=OP.add,
                        )
                        nc.sync.dma_start(out_d[128 * m : 128 * (m + 1), :], y[:])

        if loop_n is not None:
            with tc.For_i(0, loop_n, 1) as i:
                body(i)
        else:
            body()

    nc.compile()
    return nc


def prep_inputs(x, wq, bq, wk, bk, wv, bv, wp, bp, gamma, beta, wff, bff):
    """Host-side preprocessing -> per-core input maps."""
    x = np.asarray(x, dtype=np.float32)
    wq = np.asarray(wq, np.float32)
    bq = np.asarray(bq, np.float32)
    wk = np.asarray(wk, np.float32)
    wv = np.asarray(wv, np.float32)
    wp_ = np.asarray(wp, np.float32)
    bp = np.asarray(bp, np.float32)
    bv = np.asarray(bv, np.float32)
    gamma = np.asarray(gamma, np.float32)
    beta = np.asarray(beta, np.float32)
    wff = np.asarray(wff, np.float32)
    bff = np.asarray(bff, np.float32)

    scale = np.float32(1.0 / np.sqrt(np.float32(DH)))
    wq2 = np.zeros((128, 128), np.float32)
    wq2[0:64, 0:64] = wq * scale
    wq2[64:128, 64:128] = wq * scale
    wkA2 = np.zeros((128, 128), np.float32)
    wkA2[0:64, 0:64] = wk
    wkB2 = np.zeros((128, 128), np.float32)
    wkB2[64:128, 64:128] = wk
    wv2b = np.zeros((128, 130), np.float32)
    wv2b[0:64, 1:65] = wv
    wv2b[64:128, 66:130] = wv
    bq2 = (np.concatenate([bq, bq]).reshape(128, 1) * scale).astype(np.float32)
    bpp = bv @ wp_ + bp  # v-bias folded through proj
    bp2 = np.concatenate([bpp, bpp]).reshape(128, 1).astype(np.float32)
    wp2 = np.zeros((128, 128), np.float32)
    wp2[0:64, 0:64] = wp_
    wp2[64:128, 64:128] = wp_

    # channel permutation: head-major c' = h*64+dh holds original c = dh*12+h
    cp = np.arange(D)
    hh, dd = cp // 64, cp % 64
    p = dd * H + hh  # p[c'] = original channel
    wffg = wff * gamma[:, None]  # fold LN gamma into FF rows
    bffg = bff + beta @ wff  # fold LN beta through FF
    wffp = np.ascontiguousarray(wffg[p][:, p]).astype(np.float32)
    bffp = np.ascontiguousarray(
        bffg[p].reshape(NT, 128).T
    ).astype(np.float32)
    ones = np.ones((128, 1), np.float32)

    shared = {
        "wq2": wq2,
        "wkA2": wkA2,
        "wkB2": wkB2,
        "wv2b": wv2b,
        "wp2": wp2,
        "bq2": bq2,
        "bp2": bp2,
        "wffp": wffp,
        "bffp": bffp,
        "ones": ones,
    }
    in_maps = []
    for i in range(x.shape[0]):
        m = dict(shared)
        m["xT"] = np.ascontiguousarray(x[i].T)
        in_maps.append(m)
    return in_maps, p


def postprocess(results, p):
    outs = []
    for r in results:
        yt = r["out"].T  # [S, D] head-major channels
        y = np.empty_like(yt)
        y[:, p] = yt
        outs.append(y)
    return np.stack(outs)


def kernel(**inputs) -> np.ndarray:
    if "nc" not in _CACHE:
        _CACHE["nc"] = build_nc()
    nc = _CACHE["nc"]
    in_maps, p = prep_inputs(**inputs)
    res = run_bass_kernel_spmd(nc, in_maps, list(range(8)))
    return postprocess(res.results, p)



# revision 14
# speedup vs baseline: 1.3349x; 1.3349x over previous
"""Trainium2 Bass kernel for nn_MultiHeadAttn (dense transformer block).

Contract: kernel(**inputs) takes the FULL unsharded inputs from
reference.setup_inputs() and returns the FULL output [8, 1024, 768] f32.

Sharding: data-parallel over batch N=8 -> one batch item per NeuronCore
(8 cores), no collectives.

v2 design (fp16 datapath, f32 PSUM accumulation):
  - wp folded into wv on the host (attn@V@wp == attn@(V@wp)), removing the
    per-head output projection and all partition-shift DMAs.
  - bk dropped entirely (softmax is invariant to per-query score shifts).
  - One blockdiag wk2; scores for the 2 heads of a tile run as concurrent
    row-tiled matmuls (head A contracts partitions 0:64, head B 64:128).
  - AV runs as concurrent col-tiled matmuls into one [128,512] PSUM tile
    (head A -> partitions 0:64, head B -> 64:128); exp row-sums come from a
    concurrent col-tiled ones-matmul pair into a second PSUM tile.
  - exp drains PSUM->SBUF fp16 on ScalarE (the only engine with exp); the
    whole schedule is sized so ScalarE is the near-100%-busy bottleneck.
  - Normalization (1/rowsum) applied after AV (linearity), bias folded via
    per-partition tensor_scalar_add; LN gamma/beta folded into wff/bff on
    the host; output returned fp16 and upcast on the host.

Channel permutation: attention output channels are produced head-major
(c' = h*64+dh) while the module interleaves (c = dh*12+h). LN is
permutation-invariant; wff rows/cols, bff are permuted on the host and the
final output is unpermuted on the host.
"""

import numpy as np

import concourse.bacc as bacc
import concourse.mybir as mybir
import concourse.tile as tile
from concourse.bass_utils import run_bass_kernel_spmd

F32 = mybir.dt.float32
F16 = mybir.dt.float16
AF = mybir.ActivationFunctionType
OP = mybir.AluOpType

S = 1024  # sequence length
D = 768  # model dim
H = 12  # heads
DH = 64  # head dim
NT = 6  # channel tiles of 128 (2 heads each)
KC = 8  # key chunks of 128
LN_EPS = 1e-5

_CACHE = {}


def build_nc(loop_n=None, debug=False, phases="all"):
    """Build the single-core bass program (SPMD across 8 cores)."""
    nc = bacc.Bacc("TRN2", target_bir_lowering=False, debug=False)

    xT_d = nc.dram_tensor("xT", [D, S], F16, kind="ExternalInput")
    wq2_d = nc.dram_tensor("wq2", [128, 128], F16, kind="ExternalInput")
    wk2_d = nc.dram_tensor("wk2", [128, 128], F16, kind="ExternalInput")
    wv2_d = nc.dram_tensor("wv2", [128, 128], F16, kind="ExternalInput")
    bq2_d = nc.dram_tensor("bq2", [128, 1], F32, kind="ExternalInput")
    bvp2_d = nc.dram_tensor("bvp2", [128, 1], F32, kind="ExternalInput")
    ones_d = nc.dram_tensor("ones", [128, 1], F16, kind="ExternalInput")
    wffp_d = nc.dram_tensor("wffp", [D, D], F16, kind="ExternalInput")
    bffp_d = nc.dram_tensor("bffp", [128, NT], F32, kind="ExternalInput")
    out_d = nc.dram_tensor("out", [D, S], F16, kind="ExternalOutput")
    dbg = {}
    if debug:
        for nm, shp in [
            ("dbg_qT", [128, S]), ("dbg_kT", [128, S]), ("dbg_v", [128, S]),
            ("dbg_e", [128, S]), ("dbg_u", [128, 512]), ("dbg_rs", [33, 512]),
            ("dbg_a", [128, S]), ("dbg_mean", [1, S]), ("dbg_rstd", [1, S]),
            ("dbg_nr", [128, S]),
        ]:
            dbg[nm] = nc.dram_tensor(nm, shp, F32, kind="ExternalOutput")

    with tile.TileContext(nc) as tc:
        with tc.tile_pool(name="const", bufs=1) as cpool:
            # ---- constants / weights: loaded once, outside the loop ----
            def load(dram, shape, dt=F16):
                r = cpool.tile(shape, dt, name=f"r_{dram.name}")
                nc.sync.dma_start(r[:], dram[:])
                return r

            wq2r = load(wq2_d, [128, 128])
            wk2r = load(wk2_d, [128, 128])
            wv2r = load(wv2_d, [128, 128])
            onesr = load(ones_d, [128, 1])
            bq2 = load(bq2_d, [128, 1], F32)
            bvp2 = load(bvp2_d, [128, 1], F32)
            wffr = []
            for t in range(NT):
                r = cpool.tile([128, D], F16, name=f"wffr{t}")
                nc.sync.dma_start(r[:], wffp_d[128 * t : 128 * (t + 1), :])
                wffr.append(r)
            bff_all = cpool.tile([128, NT], F32, name="bff_all")
            nc.sync.dma_start(bff_all[:], bffp_d[:])
            gb = [bff_all[:, t : t + 1] for t in range(NT)]

            def body(_i=None):
                with (
                    tc.tile_pool(name="atile", bufs=1) as apool,
                    tc.tile_pool(name="psS", bufs=2, space="PSUM") as psS,
                    tc.tile_pool(name="psU", bufs=1, space="PSUM") as psU,
                    tc.tile_pool(name="psP", bufs=1, space="PSUM") as psP,
                ):
                    body_inner(apool, psS, psU, psP)

            def body_inner(apool, psS, psU, psP):
                aT = []
                with (
                    tc.tile_pool(name="qkv", bufs=1) as qkvpool,
                    tc.tile_pool(name="xr", bufs=1) as xrpool,
                    tc.tile_pool(name="p2w", bufs=2) as w2,
                ):
                    qTr = [None] * NT
                    kTr = [None] * NT
                    vsr = [None] * NT

                    def proj(t):
                        xr = xrpool.tile(
                            [128, S], F16, name="xr", tag="xr", bufs=2
                        )
                        nc.sync.dma_start(xr[:], xT_d[128 * t : 128 * (t + 1), :])
                        # q projection (+ bias, scores scale pre-folded)
                        q = qkvpool.tile([128, S], F16, name=f"qTr{t}")
                        for qh in range(2):
                            qsl = slice(512 * qh, 512 * (qh + 1))
                            psq = psP.tile([128, 512], F32, name="psq", tag="p")
                            nc.tensor.matmul(
                                psq[:], wq2r[:], xr[:, qsl],
                                start=True, stop=True,
                            )
                            nc.vector.tensor_scalar_add(q[:, qsl], psq[:], bq2[:])
                        qTr[t] = q
                        # k projection (bk dropped: softmax shift-invariant)
                        kk = qkvpool.tile([128, S], F16, name=f"kTr{t}")
                        for qh in range(2):
                            qsl = slice(512 * qh, 512 * (qh + 1))
                            psk = psP.tile([128, 512], F32, name="psk", tag="p")
                            nc.tensor.matmul(
                                psk[:], wk2r[:], xr[:, qsl],
                                start=True, stop=True,
                            )
                            nc.vector.tensor_copy(kk[:, qsl], psk[:])
                        kTr[t] = kk
                        # v' = x @ (wv @ wp): natural layout [keys, 128]
                        # (cols 0:64 head A, 64:128 head B) per 128-key chunk
                        vs = qkvpool.tile([128, S], F16, name=f"vsr{t}")
                        for vp in range(2):
                            psv = psP.tile([128, 512], F32, name="psv", tag="p")
                            for j in range(4):
                                sc = 4 * vp + j
                                nc.tensor.matmul(
                                    psv[:, 128 * j : 128 * (j + 1)],
                                    xr[:, 128 * sc : 128 * (sc + 1)],
                                    wv2r[:],
                                    start=True,
                                    stop=True,
                                )
                            nc.vector.tensor_copy(
                                vs[:, 512 * vp : 512 * (vp + 1)], psv[:]
                            )
                        vsr[t] = vs
                        if debug and t == 0:
                            nc.sync.dma_start(dbg["dbg_qT"][:], qTr[0][:])
                            nc.sync.dma_start(dbg["dbg_kT"][:], kTr[0][:])
                            nc.sync.dma_start(dbg["dbg_v"][:], vsr[0][:])

                    proj(0)
                    for t in range(NT):
                        at = apool.tile([128, S], F16, name=f"aT{t}")
                        for qh in range(2):
                            qsl = slice(512 * qh, 512 * (qh + 1))
                            u = psU.tile(
                                [128, 512], F32, name="u", tag="u", bufs=2
                            )
                            rs = psU.tile(
                                [33, 512], F32, name="rs", tag="rs", bufs=1
                            )
                            for kc in range(KC):
                                ksl = slice(128 * kc, 128 * (kc + 1))
                                sAB = psS.tile(
                                    [128, S], F32, name="sAB", tag="s"
                                )
                                # concurrent row-tiled score matmuls (2 heads)
                                nc.tensor.matmul(
                                    sAB[:, 0:512],
                                    kTr[t][0:64, ksl],
                                    qTr[t][0:64, qsl],
                                    start=True,
                                    stop=True,
                                )
                                nc.tensor.matmul(
                                    sAB[:, 512:1024],
                                    kTr[t][64:128, ksl],
                                    qTr[t][64:128, qsl],
                                    start=True,
                                    stop=True,
                                )
                                eAB = w2.tile(
                                    [128, S], F16, name="eAB", tag="e", bufs=3
                                )
                                nc.scalar.activation(eAB[:], sAB[:], AF.Exp)
                                if debug and t == 0 and qh == 0 and kc == 0:
                                    nc.sync.dma_start(dbg["dbg_e"][:], eAB[:])
                                st = kc == 0
                                fin = kc == KC - 1
                                # concurrent col-tiled AV (head A / head B)
                                nc.tensor.matmul(
                                    u[0:64, :],
                                    vsr[t][:, 128 * kc : 128 * kc + 64],
                                    eAB[:, 0:512],
                                    start=st,
                                    stop=fin,
                                    tile_position=(0, 0),
                                )
                                nc.tensor.matmul(
                                    u[64:128, :],
                                    vsr[t][:, 128 * kc + 64 : 128 * kc + 128],
                                    eAB[:, 512:1024],
                                    start=st,
                                    stop=fin,
                                    tile_position=(0, 64),
                                )
                                # concurrent col-tiled exp row-sums
                                nc.tensor.matmul(
                                    rs[0:1, :],
                                    onesr[:],
                                    eAB[:, 0:512],
                                    start=st,
                                    stop=fin,
                                    tile_position=(0, 0),
                                )
                                nc.tensor.matmul(
                                    rs[32:33, :],
                                    onesr[:],
                                    eAB[:, 512:1024],
                                    start=st,
                                    stop=fin,
                                    tile_position=(0, 32),
                                )
                                if qh == 0 and kc == 4 and t + 1 < NT:
                                    proj(t + 1)
                            if debug and t == 0 and qh == 0:
                                nc.sync.dma_start(dbg["dbg_u"][:], u[:])
                                nc.sync.dma_start(dbg["dbg_rs"][:], rs[:])
                            # normalize: a = u * (1/rowsum) + bvp2
                            rA = w2.tile([1, 512], F32, name="rA", tag="rA")
                            nc.vector.reciprocal(rA[:], rs[0:1, :])
                            rB = w2.tile([1, 512], F32, name="rB", tag="rB")
                            nc.vector.reciprocal(rB[:], rs[32:33, :])
                            rbA = w2.tile([128, 512], F32, name="rbA", tag="rbA")
                            nc.gpsimd.partition_broadcast(rbA[:], rA[:])
                            rbB = w2.tile([128, 512], F32, name="rbB", tag="rbB")
                            nc.gpsimd.partition_broadcast(rbB[:], rB[:])
                            tmp = w2.tile([128, 512], F32, name="tmp", tag="tmp")
                            nc.vector.tensor_mul(
                                tmp[0:64, :], u[0:64, :], rbA[0:64, :]
                            )
                            nc.vector.tensor_mul(
                                tmp[64:128, :], u[64:128, :], rbB[64:128, :]
                            )
                            nc.vector.tensor_scalar_add(
                                at[:, qsl], tmp[:], bvp2[:]
                            )
                        if debug and t == 0:
                            nc.sync.dma_start(dbg["dbg_a"][:], at[:])
                        aT.append(at)

                if phases == "attn":
                    for t in range(NT):
                        nc.sync.dma_start(
                            out_d[128 * t : 128 * (t + 1), :], aT[t][:]
                        )
                    return
                # ---- phase 3: LN, FF, residual ----
                with tc.tile_pool(name="p3w", bufs=2) as w3, \
                     tc.tile_pool(name="p3s", bufs=1) as s3:
                    sums = psS.tile([1, S], F32, name="sums", tag="s")
                    sumsq = psS.tile([1, S], F32, name="sumsq", tag="s")
                    for t in range(NT):
                        sq = w3.tile([128, S], F16, name="sq", tag="sq")
                        nc.vector.tensor_mul(sq[:], aT[t][:], aT[t][:])
                        st = t == 0
                        fin = t == NT - 1
                        for qh in range(2):
                            qsl = slice(512 * qh, 512 * (qh + 1))
                            nc.tensor.matmul(
                                sums[:, qsl],
                                onesr[:],
                                aT[t][:, qsl],
                                start=st,
                                stop=fin,
                            )
                            nc.tensor.matmul(
                                sumsq[:, qsl],
                                onesr[:],
                                sq[:, qsl],
                                start=st,
                                stop=fin,
                            )
                    # rstd = rsqrt(var + eps); var = sumsq/D - (sums/D)^2
                    ssb = s3.tile([1, S], F32, name="ssb")
                    nc.vector.tensor_copy(ssb[:], sums[:])
                    m2s = s3.tile([1, S], F32, name="m2s")
                    nc.vector.tensor_mul(m2s[:], ssb[:], ssb[:])
                    v768 = s3.tile([1, S], F32, name="v768")
                    nc.vector.scalar_tensor_tensor(
                        v768[:], m2s[:], -1.0 / D, sumsq[:],
                        op0=OP.mult, op1=OP.add,
                    )
                    epsr = s3.tile([1, 1], F32, name="epsr")
                    nc.vector.memset(epsr[:], LN_EPS)
                    std = s3.tile([1, S], F32, name="std")
                    nc.scalar.activation(
                        std[:], v768[:], AF.Sqrt, bias=epsr[:], scale=1.0 / D
                    )
                    rstd = s3.tile([1, S], F16, name="rstd")
                    mean = s3.tile([1, S], F16, name="mean")
                    with nc.allow_low_precision(
                        reason="fp16 LN scalars validated vs reference"
                    ):
                        nc.vector.reciprocal(rstd[:], std[:])
                        nc.vector.tensor_scalar_mul(mean[:], ssb[:], 1.0 / D)
                    meanB = s3.tile([128, S], F16, name="meanB")
                    nc.gpsimd.partition_broadcast(meanB[:], mean[:])
                    rstdB = s3.tile([128, S], F16, name="rstdB")
                    nc.gpsimd.partition_broadcast(rstdB[:], rstd[:])
                    if debug:
                        nc.sync.dma_start(dbg["dbg_mean"][:], mean[:])
                        nc.sync.dma_start(dbg["dbg_rstd"][:], rstd[:])

                    normedr = []
                    for t in range(NT):
                        d0 = w3.tile([128, S], F16, name="d0", tag="d0")
                        nc.vector.tensor_sub(d0[:], aT[t][:], meanB[:])
                        nr = s3.tile([128, S], F16, name=f"nr{t}")
                        nc.vector.tensor_mul(nr[:], d0[:], rstdB[:])
                        if debug and t == 0:
                            nc.sync.dma_start(dbg["dbg_nr"][:], nr[:])
                        normedr.append(nr)

                    for m in range(NT):
                        ff = psS.tile([128, S], F32, name="ff", tag="s")
                        for kc in range(NT):
                            st = kc == 0
                            fin = kc == NT - 1
                            for qh in range(2):
                                nc.tensor.matmul(
                                    ff[:, 512 * qh : 512 * (qh + 1)],
                                    wffr[kc][:, 128 * m : 128 * (m + 1)],
                                    normedr[kc][:, 512 * qh : 512 * (qh + 1)],
                                    start=st,
                                    stop=fin,
                                )
                        y = w3.tile([128, S], F16, name="y", tag="y")
                        nc.vector.scalar_tensor_tensor(
                            y[:], ff[:], gb[m], aT[m][:],
                            op0=OP.add, op1=OP.add,
                        )
                        nc.sync.dma_start(out_d[128 * m : 128 * (m + 1), :], y[:])

            if loop_n is not None:
                with tc.For_i(0, loop_n, 1) as i:
                    body(i)
            else:
                body()

    nc.compile()
    return nc


def prep_inputs(x, wq, bq, wk, bk, wv, bv, wp, bp, gamma, beta, wff, bff):
    """Host-side preprocessing -> per-core input maps."""
    x = np.asarray(x, dtype=np.float32)
    wq = np.asarray(wq, np.float32)
    bq = np.asarray(bq, np.float32)
    wv = np.asarray(wv, np.float32)
    wk = np.asarray(wk, np.float32)
    wp_ = np.asarray(wp, np.float32)
    bp = np.asarray(bp, np.float32)
    bv = np.asarray(bv, np.float32)
    gamma = np.asarray(gamma, np.float32)
    beta = np.asarray(beta, np.float32)
    wff = np.asarray(wff, np.float32)
    bff = np.asarray(bff, np.float32)

    scale = np.float32(1.0 / np.sqrt(np.float32(DH)))
    wq2 = np.zeros((128, 128), np.float32)
    wq2[0:64, 0:64] = wq * scale
    wq2[64:128, 64:128] = wq * scale
    wk2 = np.zeros((128, 128), np.float32)
    wk2[0:64, 0:64] = wk
    wk2[64:128, 64:128] = wk
    wvp = wv @ wp_  # proj folded through values
    wv2 = np.zeros((128, 128), np.float32)
    wv2[0:64, 0:64] = wvp
    wv2[64:128, 64:128] = wvp
    bq2 = (np.concatenate([bq, bq]).reshape(128, 1) * scale).astype(np.float32)
    bvp = bv @ wp_ + bp  # v-bias folded through proj
    bvp2 = np.concatenate([bvp, bvp]).reshape(128, 1).astype(np.float32)

    # channel permutation: head-major c' = h*64+dh holds original c = dh*12+h
    cp = np.arange(D)
    hh, dd = cp // 64, cp % 64
    p = dd * H + hh  # p[c'] = original channel
    wffg = wff * gamma[:, None]  # fold LN gamma into FF rows
    bffg = bff + beta @ wff  # fold LN beta through FF
    wffp = np.ascontiguousarray(wffg[p][:, p]).astype(np.float16)
    bffp = np.ascontiguousarray(
        bffg[p].reshape(NT, 128).T
    ).astype(np.float32)
    ones = np.ones((128, 1), np.float16)

    shared = {
        "wq2": wq2.astype(np.float16),
        "wk2": wk2.astype(np.float16),
        "wv2": wv2.astype(np.float16),
        "bq2": bq2,
        "bvp2": bvp2,
        "ones": ones,
        "wffp": wffp,
        "bffp": bffp,
    }
    in_maps = []
    for i in range(x.shape[0]):
        m = dict(shared)
        m["xT"] = np.ascontiguousarray(x[i].T).astype(np.float16)
        in_maps.append(m)
    return in_maps, p


def postprocess(results, p):
    outs = []
    for r in results:
        yt = r["out"].astype(np.float32).T  # [S, D] head-major channels
        y = np.empty_like(yt)
        y[:, p] = yt
        outs.append(y)
    return np.stack(outs)


def kernel(**inputs) -> np.ndarray:
    if "nc" not in _CACHE:
        _CACHE["nc"] = build_nc()
    nc = _CACHE["nc"]
    in_maps, p = prep_inputs(**inputs)
    res = run_bass_kernel_spmd(nc, in_maps, list(range(8)))
    return postprocess(res.results, p)
